# revision 9
# baseline (speedup 1.0000x reference)
"""Trainium2 Bass kernel for nn_Conv2dGeneral (capsule-style 4x4-pose conv).

Math (from the reference):
  out[b,o,X,Y,u,w] = sum_{cin,kx,ky,v} Wm[(cin,kx,ky),o,u,v] * x[b,cin,2X+kx,2Y+ky,4v+w] + bias[o]

Mapped to the PE array as a single 1152-deep contraction:
  K = (cin, v)  x  9 accumulation chunks over (kx, ky)   [9 x 128]
  M = (o, u)                                              [128 PSUM partitions]
  N = (X, Y, w)                                           [676 per batch image]

Data-parallel across 8 NeuronCores on the batch dim (8 images per core).

Host-side prep: x is re-laid-out to [(b), (cin,v), (r,c,w)] so each core's
shard DMAs as fully-contiguous partition lines; the stride-2 im2col window
gather then happens for free inside the matmul moving-operand access
pattern (no patch materialization, each x byte read once from HBM).

v2 overlap schedule:
  - PE warms its HAM clock gate on a DVE-memset tile starting right after
    the framework preamble (no DMA dependency), so the 1.2->2.4 GHz ramp
    is done before the first real matmul.
  - DMA issue order: W first (small, needed by the first LDWEIGHTS), then
    the 8 images back-to-back; bias rides the ACT HWDGE queue instead of
    taking an issue slot in the x stream.
  - Output staged fp16 (error << gate; fp32 accumulation in PSUM), shipped
    per-image right after its second half is evicted -> short DMA tail.
"""

import numpy as np

B, CIN, COUT = 64, 32, 32
KK, STRIDE = 3, 2
WIN, HH = 28, 16
H = 4
WOUT = (WIN - KK) // STRIDE + 1  # 13
NCORES = 8
BPC = B // NCORES                # batches per core
RCW = WIN * WIN * H              # 3136 free elements per (cin,v) partition
                                 # (28x28 kept: a 27x27 slice breaks the 32B
                                 # stride phase of the PE moving-operand reads
                                 # and costs ~14% matmul cadence)
NOUT = WOUT * WOUT * H           # 676 outputs per (o,u) partition per image
XSPLIT = ((0, 7), (7, 6))        # two PSUM groups: X rows [0,7) and [7,13)
WARMUP = 30                      # PE warmup matmuls: engines unblock ~7.3us
                                 # (framework preamble barrier), x0 lands
                                 # ~12.4us -> ~3.2us of cold matmuls bridge

_cache = {}


def _build_bass():
    """Raw-bass build (no Tile): this toolchain's walrus codegen allows only
    ONE sync-wait per instruction, so all cross-engine sync is explicit
    single-sem waits; ordering beyond that rides on hardware transitivity.

    Engines: SP triggers the input DMAs, DVE memsets the warmup tile, PE
    runs 16 accumulation groups of 9 matmuls (one per kernel tap), ACT
    evicts PSUM->SBUF (adding bias, casting to fp16) and ships each image.
    """
    import concourse.bass as bass
    import concourse.mybir as mybir

    f32 = mybir.dt.float32
    f16 = mybir.dt.float16
    NG = 2 * BPC              # 16 PSUM accumulation groups

    nc = bass.Bass()
    x_d = nc.declare_dram_parameter("x", [BPC, 128, RCW], f16, isOutput=False)
    w_d = nc.declare_dram_parameter("w", [128, 9 * 128], f16, isOutput=False)
    b_d = nc.declare_dram_parameter("b", [128, 1], f32, isOutput=False)
    o_d = nc.declare_dram_parameter("out", [BPC, 128, NOUT], f16, isOutput=True)

    with (
        nc.sbuf_tensor([128, 9 * 128], f16) as wt,
        nc.sbuf_tensor([128, 1], f32) as bt,
        nc.sbuf_tensor([128, 128], f16) as wu,
        nc.sbuf_tensor([128, BPC, RCW], f16) as gt,
        nc.sbuf_tensor([128, BPC, NOUT], f16) as ot,
        nc.psum_tensor([128, 8, 512], f32) as ps,
        nc.semaphore("wt_sem") as wt_sem,
        nc.semaphore("bias_sem") as bias_sem,
        nc.semaphore("wu_sem") as wu_sem,
        nc.semaphore("g_sem0") as g_sem0,
        nc.semaphore("g_sem1") as g_sem1,
        nc.semaphore("g_sem2") as g_sem2,
        nc.semaphore("g_sem3") as g_sem3,
        nc.semaphore("g_sem4") as g_sem4,
        nc.semaphore("g_sem5") as g_sem5,
        nc.semaphore("g_sem6") as g_sem6,
        nc.semaphore("g_sem7") as g_sem7,
        nc.semaphore("pe_sem") as pe_sem,
        nc.semaphore("act_sem") as act_sem,
        nc.semaphore("out_sem") as out_sem,
        nc.Block() as block,
    ):
        g_sems = [g_sem0, g_sem1, g_sem2, g_sem3, g_sem4, g_sem5, g_sem6, g_sem7]
        wtr = wt[:, :].rearrange("p (k m) -> p k m", k=9)

        @block.vector
        def _(vector):
            vector.memset(wu[:, :], 0.0).then_inc(wu_sem, 1)

        @block.sync
        def _(sync):
            # W first: the first real LDWEIGHTS gates on it and it's small.
            sync.dma_start(wt[:, :], w_d[:, :]).then_inc(wt_sem, 16)
            # One DMA per image per semaphore: a sem shared by two DMAs can
            # hit >=16 from a subset of fast SDMA engines finishing both
            # chunks while slow engines still stream the first (observed
            # race -> NaNs), so chunked gating is not sound with one sem.
            for b in range(BPC):
                sync.dma_start(gt[:, b, :], x_d[b]).then_inc(g_sems[b], 16)
            sync.wait_ge(out_sem, 16 * BPC)

        @block.tensor
        def _(tensor):
            # Warm the PE HAM clock gate (cold = 1.2 GHz) on the zeroed DVE
            # tile while W/x stream in; ~32 cold matmuls flip it to 2.4 GHz,
            # the rest keep it busy until the first x chunk lands.
            tensor.wait_ge(wu_sem, 1)
            for i in range(WARMUP):
                tensor.matmul(
                    ps[:, 7, :128], wu[:, :], wu[:, :], start=True, stop=True
                )
            tensor.wait_ge(wt_sem, 16)
            for j in range(NG):
                b, half = divmod(j, 2)
                if half == 0:
                    tensor.wait_ge(g_sems[b], 16)
                if j >= 8:
                    # PSUM bank j%8 is free once ACT drained group j-8
                    tensor.wait_ge(act_sem, j - 7)
                X0, nX = XSPLIT[half]
                gr = gt[:, b, :].rearrange("p (r c w) -> p r c w", r=WIN, c=WIN)
                for kk in range(9):
                    kx, ky = divmod(kk, 3)
                    rhs = gr[
                        :,
                        2 * X0 + kx : 2 * X0 + kx + 2 * nX - 1 : 2,
                        ky : ky + 2 * WOUT - 1 : 2,
                        :,
                    ]
                    mm = tensor.matmul(
                        ps[:, j % 8, : nX * WOUT * H],
                        wtr[:, kk, :],
                        rhs,
                        start=(kk == 0),
                        stop=(kk == 8),
                    )
                mm.then_inc(pe_sem, 1)

        @block.scalar
        def _(scalar):
            # bias rides the ACT HWDGE queue: keeps the Sync queue a pure x
            # stream (each issue slot there is ~0.65us) and ACT only needs
            # it ~13us in.
            scalar.dma_start(bt[:, :], b_d[:, :]).then_inc(bias_sem, 16)
            scalar.wait_ge(bias_sem, 16)
            for j in range(NG):
                b, half = divmod(j, 2)
                X0, nX = XSPLIT[half]
                off = X0 * WOUT * H
                scalar.wait_ge(pe_sem, j + 1)
                scalar.activation(
                    ot[:, b, off : off + nX * WOUT * H],
                    ps[:, j % 8, : nX * WOUT * H],
                    mybir.ActivationFunctionType.Identity,
                    bias=bt[:, :],
                ).then_inc(act_sem, 1)
                if half == 1:
                    # image complete; ship it from the ACT ring
                    scalar.dma_start(o_d[b], ot[:, b, :]).then_inc(out_sem, 16)

    return nc


def _prep_inputs(x, W, bias):
    # x: (B, CIN, 28, 28, 16) -> xp[b, cin*4+v, (r*28+c)*4+w] = x[b,cin,r,c,4v+w]
    # fp16: PE runs fp32 matmuls as LOW_HIGH double passes; fp16 is single-pass
    # with fast-weight-load, and halves the dominant HBM traffic. Max rel err
    # ~3e-4 at this contraction depth (fp32 PSUM accumulation).
    xp = np.ascontiguousarray(
        x.reshape(B, CIN, WIN, WIN, H, H).transpose(0, 1, 4, 2, 3, 5)
    ).reshape(B, CIN * H, RCW).astype(np.float16)
    # W: (1, 288, 32, 1, 1, 4, 4); p = cin*9 + kx*3 + ky
    # wt_sb[cin*4+v, kk*128 + o*4+u] = Wm[cin*9+kk, o, u, v]
    Wm = np.asarray(W, dtype=np.float32).reshape(CIN, KK * KK, COUT, H, H)
    wt_sb = np.ascontiguousarray(
        Wm.transpose(0, 4, 1, 2, 3)  # cin, v, kk, o, u
    ).reshape(128, 9 * 128).astype(np.float16)
    bias_v = np.ascontiguousarray(
        np.repeat(np.asarray(bias, dtype=np.float32).reshape(COUT), H)
    ).reshape(128, 1)
    return xp, wt_sb, bias_v


def _shard_x(xp, core):
    # per-core input: [BPC, 128, RCW] fp16
    return np.ascontiguousarray(xp[core * BPC : (core + 1) * BPC])


def _unprep_output(full):
    # full: (B, 128, NOUT) with partition o*4+u, free (X, Y, w)
    out = (
        full.astype(np.float32)
        .reshape(B, COUT, H, WOUT, WOUT, H)
        .transpose(0, 1, 3, 4, 2, 5)
        .reshape(B, COUT, WOUT, WOUT, HH)
    )
    return np.ascontiguousarray(out)


def run_device(in_maps, trace=False, tmpdir=None):
    from concourse.bass_utils import run_bass_kernel_spmd

    if "nc" not in _cache:
        _cache["nc"] = _build_bass()
    return run_bass_kernel_spmd(
        _cache["nc"], in_maps, list(range(NCORES)), trace=trace, tmpdir=tmpdir
    )


def kernel(x, W, bias):
    x = np.asarray(x, dtype=np.float32)
    xp, wt_sb, bias_v = _prep_inputs(x, W, bias)
    in_maps = [
        {"x": _shard_x(xp, i), "w": wt_sb, "b": bias_v} for i in range(NCORES)
    ]
    res = run_device(in_maps, trace=False)
    full = np.concatenate(
        [res.results[i]["out"] for i in range(NCORES)], axis=0
    )
    return _unprep_output(full)
